# revision 24
# baseline (speedup 1.0000x reference)
"""Multi-head attention (B=2, S=2048, D=1024, H=16) on 8 TRN2 NeuronCores.

Sharding: data-parallel over batch (2) x tensor-parallel over head groups
(4 groups of 4 heads).  Core c = (b = c // 4, g = c % 4).  Each core:
  q/k/v = x[b] @ W{q,k,v}[:, 256g:256g+256] + b{q,k,v}[...]   (1/sqrt(dh)
  folded into Wq/bq on host), per-head softmax(q k^T) v, then a partial
  out-projection y_c = attn_out @ Wo[256g:256g+256, :].  Host sums the 4
  partials per batch and adds bo.

v2 design notes (vs the fp32r baseline):
  * all matmul operands bf16 (halves DMA; rel err ~6e-3 vs 2e-2 gate)
  * attention inner loop software-pipelined: scores(kt+2) + exp(kt+2)
    emitted ahead of pv(kt), so the PE never drains and stays at the
    2.4 GHz p-state; ACT (exp) streams back-to-back.
  * projections / out-projection matmuls are interleaved as PE "filler"
    inside the attention stream to cover the PE-vs-ACT rate gap.
  * q/k bias adds moved from ACT (activation Identity) to DVE
    tensor_scalar_add; softmax denominators normalized via
    reciprocal_approx_fast (1 DVE op) straight off the PV psum row,
    broadcast on gpsimd, applied late (deferred normalization) so the
    single PV psum buffer is freed by two cheap copies.
  * out-projection DMAs straight from PSUM (no DVE staging copy).

Device layouts (per core):
  xT   [1024, 2048]  (x[b] transposed on host, bf16)
  qT/kT: [256, 2048] as 2 sbuf tiles [128, 2048] (head h -> partitions
         64*(h%2).. of tile h//2)
  v_ext: 16 tiles [128, 260]; head h at cols 65h..65h+63, ones at 65h+64
         (ones column makes P @ V_ext also emit softmax denominators)
  scores^T per (head, ktile): [128, 1024] per q-half
  uT/aT: 2 tiles [128, 2048] (unnormalized / normalized attn out^T)
  yT   [1024, 2048] partial output (ExternalOutput, f32)
"""

import os
import sys
import types
from contextlib import ExitStack

import numpy as np

D = 1024
S = 2048
C = 256          # head cols per core (4 heads x 64)
DH = 64
NH = 4           # heads per core
QHW = 1024       # q-half width

_CACHE = {}


def _install_ntff_shim():
    try:
        import antenv.axon_hooks  # noqa: F401
        return
    except ImportError:
        pass
    try:
        from trn_agent_boot.trn_boot import _ntff_profile_via_ctypes
        hook = _ntff_profile_via_ctypes('/opt/axon/libaxon_pjrt.so')
    except Exception:
        hook = None
    mod = types.ModuleType('antenv.axon_hooks')
    mod.get_axon_ntff_profile_hook = lambda: hook
    mod.set_axon_ntff_profile_hook = lambda h: None
    sys.modules['antenv.axon_hooks'] = mod


def build_nc(seq=S):
    import concourse.bacc as bacc
    import concourse.mybir as mybir
    import concourse.tile as tile
    from concourse.bass import ts, ds

    F32 = mybir.dt.float32
    BF = mybir.dt.bfloat16
    ACT = mybir.ActivationFunctionType

    nst = seq // 128          # 128-row k tiles of seq
    nqb = seq // 512          # 512-wide column blocks of seq
    qh_w = min(QHW, seq)      # q-half width
    nqh = seq // qh_w         # number of q halves
    qh_b = qh_w // 512        # 512-blocks per q half

    dbg = bool(os.environ.get("KERNEL_DBG"))
    nc = bacc.Bacc("TRN2", target_bir_lowering=False, debug=False)
    xT = nc.dram_tensor("xT", [D, seq], BF, kind="ExternalInput")
    wqkv = nc.dram_tensor("wqkv", [D, 3 * C], BF, kind="ExternalInput")
    wo = nc.dram_tensor("wo", [C, D], BF, kind="ExternalInput")
    bqk = nc.dram_tensor("bqk", [128, 4], F32, kind="ExternalInput")  # [bq0 bq1 bk0 bk1]
    bv = nc.dram_tensor("bv", [1, C], F32, kind="ExternalInput")
    yT = nc.dram_tensor("yT", [D, seq], F32, kind="ExternalOutput")
    if dbg:
        d_qT = nc.dram_tensor("d_qT", [C, seq], F32, kind="ExternalOutput")
        d_kT = nc.dram_tensor("d_kT", [C, seq], F32, kind="ExternalOutput")
        d_v = nc.dram_tensor("d_v", [seq, NH * 65], F32, kind="ExternalOutput")
        d_uT = nc.dram_tensor("d_uT", [C, seq], F32, kind="ExternalOutput")
        d_aT = nc.dram_tensor("d_aT", [C, seq], F32, kind="ExternalOutput")
        d_rn = nc.dram_tensor("d_rn", [NH, seq], F32, kind="ExternalOutput")

    with tile.TileContext(nc) as tc, ExitStack() as ctx:
        consts = ctx.enter_context(tc.tile_pool(name="consts", bufs=1))
        sbw = ctx.enter_context(tc.tile_pool(name="weights", bufs=1))
        sbx = ctx.enter_context(tc.tile_pool(name="xT", bufs=1))
        sba = ctx.enter_context(tc.tile_pool(name="acts", bufs=1))
        ptp = ctx.enter_context(tc.tile_pool(name="pt", bufs=4))
        rbcp = ctx.enter_context(tc.tile_pool(name="rbc", bufs=2))
        syp = ctx.enter_context(tc.tile_pool(name="syp", bufs=4))
        # PSUM: smp 2x1 + scp 2x2 + pvp 1x2 = 8 banks
        smp = ctx.enter_context(tc.tile_pool(name="smp", bufs=2, space="PSUM"))
        scp = ctx.enter_context(tc.tile_pool(name="scp", bufs=2, space="PSUM"))
        pvp = ctx.enter_context(tc.tile_pool(name="pvp", bufs=1, space="PSUM"))

        # ---- constants ----
        bqk_sb = consts.tile([128, 4], F32, tag="bqk", name="bqk_sb")
        nc.sync.dma_start(bqk_sb[:], bqk[:, :])
        bv_row = consts.tile([1, C], F32, tag="bvrow", name="bv_row")
        nc.sync.dma_start(bv_row[:], bv[:, :])
        bvb = consts.tile([128, C], F32, tag="bvb", name="bvb")
        nc.gpsimd.partition_broadcast(bvb[:], bv_row[:])
        ones4 = consts.tile([128, NH], BF, tag="ones4", name="ones4")
        nc.vector.memset(ones4[:], 1.0)

        # ---- loads: few, fat DMAs (trigger issue on the sync queue costs
        #      ~600ns each, so 26 triggers instead of 92) ----
        xt_sb = [sbx.tile([128, seq], BF, tag=f"xt{i}", name=f"xt{i}")
                 for i in range(8)]
        w_sb = []
        for i in range(8):
            t = sbw.tile([128, 3 * C], BF, tag=f"w{i}", name=f"w{i}")
            nc.sync.dma_start(t[:], wqkv[ts(i, 128), :])
            w_sb.append(t)
        for half in range(2):
            for i in range(8):
                nc.sync.dma_start(xt_sb[i][:, ts(half, 1024)],
                                  xT[ts(i, 128), ts(half, 1024)])
        wo_sb = []
        for i in range(2):
            t = sbw.tile([128, D], BF, tag=f"wo{i}", name=f"wo{i}")
            nc.sync.dma_start(t[:], wo[ts(i, 128), :])
            wo_sb.append(t)
        w_off = {"q": 0, "k": C, "v": 2 * C}

        # ---- persistent activations ----
        qT_sb = [sba.tile([128, seq], BF, tag=f"qT{i}", name=f"qT{i}") for i in range(2)]
        kT_sb = [sba.tile([128, seq], BF, tag=f"kT{i}", name=f"kT{i}") for i in range(2)]
        v_sb = [sba.tile([128, NH * 65], BF, tag=f"v{i}", name=f"v{i}") for i in range(nst)]
        uT_sb = [sba.tile([128, seq], F32, tag=f"uT{i}", name=f"uT{i}") for i in range(2)]
        aT_sb = [sba.tile([128, seq], BF, tag=f"aT{i}", name=f"aT{i}") for i in range(2)]
        rn = [sba.tile([1, seq], F32, tag=f"rn{h}", name=f"rn{h}")
              for h in range(NH)]

        # ---- emission helpers (each emits ~one PSUM tile of PE work) ----
        def emit_qk(name, bias_col, mt, nb):
            dst = qT_sb if name == "q" else kT_sb
            ps = smp.tile([128, 512], F32, tag="sm", name="ps")
            for kt in range(8):
                nc.tensor.matmul(
                    ps[:],
                    lhsT=w_sb[kt][:, ds(w_off[name] + mt * 128, 128)],
                    rhs=xt_sb[kt][:, ts(nb, 512)],
                    start=(kt == 0), stop=(kt == 7))
            nc.vector.tensor_scalar_add(
                dst[mt][:, ts(nb, 512)], ps[:],
                bqk_sb[:, bias_col + mt:bias_col + mt + 1])

        def emit_v(st):
            ps = smp.tile([128, C], F32, tag="sm", name="vps")
            for kt in range(8):
                nc.tensor.matmul(
                    ps[:], lhsT=xt_sb[kt][:, ts(st, 128)],
                    rhs=w_sb[kt][:, ds(2 * C, C)],
                    start=(kt == 0), stop=(kt == 7))
            v3 = v_sb[st][:].rearrange("p (h e) -> p h e", e=65)
            nc.vector.tensor_copy(
                v3[:, :, 64:65],
                ones4[:].rearrange("p (h e) -> p h e", e=1))
            nc.vector.tensor_add(
                v3[:, :, 0:64],
                ps[:].rearrange("p (h e) -> p h e", e=64),
                bvb[:].rearrange("p (h e) -> p h e", e=64))

        def emit_out(qh, i, alt=False):
            mt, nbl = divmod(i, qh_b)
            nb = qh * qh_b + nbl
            # after attention finishes, scp's banks are free: alternate the
            # final out-proj tiles between pools for a 4-deep PSUM pipeline
            if alt and i % 2:
                yp = scp.tile([128, 512], F32, tag="sc", name="yp")
            else:
                yp = smp.tile([128, 512], F32, tag="sm", name="yp")
            for kt2 in range(2):
                nc.tensor.matmul(
                    yp[:], lhsT=wo_sb[kt2][:, ts(mt, 128)],
                    rhs=aT_sb[kt2][:, ts(nb, 512)],
                    start=(kt2 == 0), stop=(kt2 == 1))
            yt = syp.tile([128, 512], F32, tag="yt", name="yt")
            nc.vector.tensor_copy(yt[:], yp[:])
            nc.sync.dma_start(yT[ts(mt, 128), ts(nb, 512)], yt[:])

        def attn_head(qh, h, fillers=()):
            fillers = list(fillers)
            tidx, poff = h // 2, 64 * (h % 2)
            qt, ktt = qT_sb[tidx], kT_sb[tidx]
            pv = pvp.tile([65, qh_w], F32, tag="pv", name="pv")
            pts = {}

            def emit_sc(kt):
                sc = scp.tile([128, qh_w], F32, tag="sc", name="sc")
                for qb in range(qh_b):
                    nc.tensor.matmul(
                        sc[:, ts(qb, 512)],
                        lhsT=ktt[poff:poff + 64, ts(kt, 128)],
                        rhs=qt[poff:poff + 64, ds(qh * qh_w + qb * 512, 512)],
                        start=True, stop=True)
                pt = ptp.tile([128, qh_w], BF, tag="pt", name="pt")
                nc.scalar.activation(pt[:], sc[:], ACT.Exp)
                pts[kt] = pt

            emit_sc(0)
            emit_sc(1)
            fi = 0
            for kt in range(nst):
                # filler first: the PE chews it while waiting on exp(kt)
                n_due = (len(fillers) * (kt + 1) + nst - 1) // nst
                while fi < n_due:
                    fillers[fi]()
                    fi += 1
                if kt + 2 < nst:
                    emit_sc(kt + 2)
                pt = pts.pop(kt)
                for qb in range(qh_b):
                    nc.tensor.matmul(
                        pv[:, ts(qb, 512)],
                        lhsT=v_sb[kt][:, ds(65 * h, 65)],
                        rhs=pt[:, ts(qb, 512)],
                        start=(kt == 0), stop=(kt == nst - 1))
            while fi < len(fillers):
                fillers[fi]()
                fi += 1
            # drain pv: denominator chain first (it gates broadcast->mul->out),
            # then the unnormalized-out copy.  reciprocal_approx_fast misreads
            # PSUM inputs, so stage the denominator row through SBUF.
            dr = sba.tile([1, qh_w], F32, tag=f"dr{h}", name=f"dr{h}")
            nc.vector.tensor_copy(dr[:], pv[64:65, :])
            nc.vector.reciprocal_approx_fast(
                rn[h][:, ds(qh * qh_w, qh_w)], dr[:])
            nc.vector.tensor_copy(
                uT_sb[tidx][poff:poff + 64, ds(qh * qh_w, qh_w)], pv[0:64, :])

        def norm_head(qh, h):
            # broadcast to all 128 partitions (partition_broadcast only
            # writes starting at partition 0); the mul then reads the 64-row
            # half at the same base partition as uT/aT (engine constraint:
            # SBUF-SBUF TensorTensor needs equal base partitions)
            tidx, j = h // 2, h % 2
            sl = ds(qh * qh_w, qh_w)
            rb = rbcp.tile([128, qh_w], F32, tag=f"rb{j}", name=f"rb{j}")
            nc.gpsimd.partition_broadcast(rb[:], rn[h][:, sl])
            half = slice(64 * j, 64 * j + 64)
            nc.vector.tensor_mul(
                aT_sb[tidx][half, sl], uT_sb[tidx][half, sl], rb[half, :])

        # ---------------- schedule ----------------
        for nb in range(nqb):
            emit_qk("q", 0, 0, nb)
        for nb in range(nqb):
            emit_qk("k", 2, 0, nb)
        attn_head(0, 0, [lambda st=st: emit_v(st) for st in range(nst)])
        norm_head(0, 0)
        attn_head(0, 1,
                  [lambda nb=nb: emit_qk("q", 0, 1, nb) for nb in range(nqb)] +
                  [lambda nb=nb: emit_qk("k", 2, 1, nb) for nb in range(nqb)])
        norm_head(0, 1)
        attn_head(0, 2)
        norm_head(0, 2)
        attn_head(0, 3)
        norm_head(0, 3)
        attn_head(1, 0, [lambda i=i: emit_out(0, i) for i in range(0, 4)])
        norm_head(1, 0)
        attn_head(1, 1, [lambda i=i: emit_out(0, i) for i in range(4, 8)])
        norm_head(1, 1)
        attn_head(1, 2, [lambda i=i: emit_out(0, i) for i in range(8, 12)])
        norm_head(1, 2)
        attn_head(1, 3, [lambda i=i: emit_out(0, i) for i in range(12, 16)])
        norm_head(1, 3)
        for i in range(16):
            emit_out(1, i, alt=True)

        if dbg:
            dpool = ctx.enter_context(tc.tile_pool(name="dbg", bufs=2))
            def dump(dram, tiles, width):
                for i, t in enumerate(tiles):
                    for c0 in range(0, width, 1024):
                        w = min(1024, width - c0)
                        dt_ = dpool.tile([128, 1024], F32, tag="d", name="dt")
                        nc.vector.tensor_copy(dt_[:, :w], t[:, c0:c0 + w])
                        nc.sync.dma_start(dram[ts(i, 128), c0:c0 + w],
                                          dt_[:, :w])
            dump(d_qT, qT_sb, seq)
            dump(d_kT, kT_sb, seq)
            dump(d_v, v_sb, NH * 65)
            dump(d_uT, uT_sb, seq)
            dump(d_aT, aT_sb, seq)
            for h in range(NH):
                dt_ = dpool.tile([1, seq], F32, tag="dr", name="dtr")
                nc.vector.tensor_copy(dt_[:], rn[h][:])
                nc.sync.dma_start(d_rn[h:h + 1, :], dt_[:])

    nc.compile()
    return nc


def make_in_maps(x, Wq, bq, Wk, bk, Wv, bv, Wo):
    """Shard full inputs into 8 per-core input maps."""
    import ml_dtypes
    BF = ml_dtypes.bfloat16
    scale = np.float32(1.0 / np.sqrt(DH))
    xT = [np.ascontiguousarray(x[b].T).astype(BF) for b in range(2)]
    in_maps = []
    for c in range(8):
        b, g = c // 4, c % 4
        sl = slice(C * g, C * (g + 1))
        bq_g = (bq[sl] * scale).reshape(2, 128).T
        bk_g = bk[sl].reshape(2, 128).T
        in_maps.append({
            "xT": xT[b],
            "wqkv": np.ascontiguousarray(np.concatenate(
                [Wq[:, sl] * scale, Wk[:, sl], Wv[:, sl]], axis=1)).astype(BF),
            "wo": np.ascontiguousarray(Wo[sl, :]).astype(BF),
            "bqk": np.ascontiguousarray(
                np.concatenate([bq_g, bk_g], axis=1)).astype(np.float32),
            "bv": bv[sl].reshape(1, C).astype(np.float32),
        })
    return in_maps


def kernel(x, Wq, bq, Wk, bk, Wv, bv, Wo, bo):
    if os.environ.get("JAX_PLATFORMS") and \
            "axon" not in os.environ["JAX_PLATFORMS"]:
        os.environ.pop("JAX_PLATFORMS")
    trace = bool(os.environ.get("KERNEL_TRACE"))
    if trace:
        _install_ntff_shim()
    from concourse import bass_utils

    x = np.asarray(x, dtype=np.float32)
    in_maps = make_in_maps(
        x, np.asarray(Wq), np.asarray(bq), np.asarray(Wk), np.asarray(bk),
        np.asarray(Wv), np.asarray(bv), np.asarray(Wo))

    if "nc" not in _CACHE:
        _CACHE["nc"] = build_nc()
    res = bass_utils.run_bass_kernel_spmd(
        _CACHE["nc"], in_maps, core_ids=list(range(8)), trace=trace)
    _CACHE["exec_time_ns"] = res.exec_time_ns

    bo = np.asarray(bo, dtype=np.float32)
    out = np.empty((2, S, D), dtype=np.float32)
    for b in range(2):
        acc = res.results[4 * b]["yT"].copy()
        for g in range(1, 4):
            acc += res.results[4 * b + g]["yT"]
        out[b] = acc.T + bo
    return out


# revision 25
# speedup vs baseline: 1.0151x; 1.0151x over previous
"""Multi-head attention (B=2, S=2048, D=1024, H=16) on 8 TRN2 NeuronCores.

Sharding: data-parallel over batch (2) x tensor-parallel over head groups
(4 groups of 4 heads).  Core c = (b = c // 4, g = c % 4).  Each core:
  q/k/v = x[b] @ W{q,k,v}[:, 256g:256g+256] + b{q,k,v}[...]   (1/sqrt(dh)
  folded into Wq/bq on host), per-head softmax(q k^T) v, then a partial
  out-projection y_c = attn_out @ Wo[256g:256g+256, :].  Host sums the 4
  partials per batch and adds bo.

v2 design notes (vs the fp32r baseline):
  * all matmul operands bf16 (halves DMA; rel err ~6e-3 vs 2e-2 gate)
  * attention inner loop software-pipelined: scores(kt+2) + exp(kt+2)
    emitted ahead of pv(kt), so the PE never drains and stays at the
    2.4 GHz p-state; ACT (exp) streams back-to-back.
  * projections / out-projection matmuls are interleaved as PE "filler"
    inside the attention stream to cover the PE-vs-ACT rate gap.
  * q/k bias adds moved from ACT (activation Identity) to DVE
    tensor_scalar_add; softmax denominators normalized via
    reciprocal_approx_fast (1 DVE op) straight off the PV psum row,
    broadcast on gpsimd, applied late (deferred normalization) so the
    single PV psum buffer is freed by two cheap copies.
  * out-projection DMAs straight from PSUM (no DVE staging copy).

Device layouts (per core):
  xT   [1024, 2048]  (x[b] transposed on host, bf16)
  qT/kT: [256, 2048] as 2 sbuf tiles [128, 2048] (head h -> partitions
         64*(h%2).. of tile h//2)
  v_ext: 16 tiles [128, 260]; head h at cols 65h..65h+63, ones at 65h+64
         (ones column makes P @ V_ext also emit softmax denominators)
  scores^T per (head, ktile): [128, 1024] per q-half
  uT/aT: 2 tiles [128, 2048] (unnormalized / normalized attn out^T)
  yT   [1024, 2048] partial output (ExternalOutput, f32)
"""

import os
import sys
import types
from contextlib import ExitStack

import numpy as np

D = 1024
S = 2048
C = 256          # head cols per core (4 heads x 64)
DH = 64
NH = 4           # heads per core
QHW = 1024       # q-half width

_CACHE = {}


def _install_ntff_shim():
    try:
        import antenv.axon_hooks  # noqa: F401
        return
    except ImportError:
        pass
    try:
        from trn_agent_boot.trn_boot import _ntff_profile_via_ctypes
        hook = _ntff_profile_via_ctypes('/opt/axon/libaxon_pjrt.so')
    except Exception:
        hook = None
    mod = types.ModuleType('antenv.axon_hooks')
    mod.get_axon_ntff_profile_hook = lambda: hook
    mod.set_axon_ntff_profile_hook = lambda h: None
    sys.modules['antenv.axon_hooks'] = mod


def build_nc(seq=S):
    import concourse.bacc as bacc
    import concourse.mybir as mybir
    import concourse.tile as tile
    from concourse.bass import ts, ds

    F32 = mybir.dt.float32
    BF = mybir.dt.bfloat16
    ACT = mybir.ActivationFunctionType

    nst = seq // 128          # 128-row k tiles of seq
    nqb = seq // 512          # 512-wide column blocks of seq
    qh_w = min(QHW, seq)      # q-half width
    nqh = seq // qh_w         # number of q halves
    qh_b = qh_w // 512        # 512-blocks per q half

    dbg = bool(os.environ.get("KERNEL_DBG"))
    nc = bacc.Bacc("TRN2", target_bir_lowering=False, debug=False)
    xT = nc.dram_tensor("xT", [D, seq], BF, kind="ExternalInput")
    wqkv = nc.dram_tensor("wqkv", [D, 3 * C], BF, kind="ExternalInput")
    wo = nc.dram_tensor("wo", [C, D], BF, kind="ExternalInput")
    bqk = nc.dram_tensor("bqk", [128, 4], F32, kind="ExternalInput")  # [bq0 bq1 bk0 bk1]
    bv = nc.dram_tensor("bv", [1, C], F32, kind="ExternalInput")
    yT = nc.dram_tensor("yT", [D, seq], F32, kind="ExternalOutput")
    if dbg:
        d_qT = nc.dram_tensor("d_qT", [C, seq], F32, kind="ExternalOutput")
        d_kT = nc.dram_tensor("d_kT", [C, seq], F32, kind="ExternalOutput")
        d_v = nc.dram_tensor("d_v", [seq, NH * 65], F32, kind="ExternalOutput")
        d_uT = nc.dram_tensor("d_uT", [C, seq], F32, kind="ExternalOutput")
        d_aT = nc.dram_tensor("d_aT", [C, seq], F32, kind="ExternalOutput")
        d_rn = nc.dram_tensor("d_rn", [NH, seq], F32, kind="ExternalOutput")

    with tile.TileContext(nc) as tc, ExitStack() as ctx:
        consts = ctx.enter_context(tc.tile_pool(name="consts", bufs=1))
        sbw = ctx.enter_context(tc.tile_pool(name="weights", bufs=1))
        sbx = ctx.enter_context(tc.tile_pool(name="xT", bufs=1))
        sba = ctx.enter_context(tc.tile_pool(name="acts", bufs=1))
        ptp = ctx.enter_context(tc.tile_pool(name="pt", bufs=4))
        rbcp = ctx.enter_context(tc.tile_pool(name="rbc", bufs=2))
        syp = ctx.enter_context(tc.tile_pool(name="syp", bufs=4))
        # PSUM: smp 2x1 + scp 2x2 + pvp 1x2 = 8 banks
        smp = ctx.enter_context(tc.tile_pool(name="smp", bufs=2, space="PSUM"))
        scp = ctx.enter_context(tc.tile_pool(name="scp", bufs=2, space="PSUM"))
        pvp = ctx.enter_context(tc.tile_pool(name="pvp", bufs=1, space="PSUM"))

        # ---- constants ----
        bqk_sb = consts.tile([128, 4], F32, tag="bqk", name="bqk_sb")
        nc.sync.dma_start(bqk_sb[:], bqk[:, :])
        bv_row = consts.tile([1, C], F32, tag="bvrow", name="bv_row")
        nc.sync.dma_start(bv_row[:], bv[:, :])
        bvb = consts.tile([128, C], F32, tag="bvb", name="bvb")
        nc.gpsimd.partition_broadcast(bvb[:], bv_row[:])
        ones4 = consts.tile([128, NH], BF, tag="ones4", name="ones4")
        nc.vector.memset(ones4[:], 1.0)

        # ---- loads: few, fat DMAs (trigger issue on the sync queue costs
        #      ~600ns each, so 26 triggers instead of 92) ----
        xt_sb = [sbx.tile([128, seq], BF, tag=f"xt{i}", name=f"xt{i}")
                 for i in range(8)]
        w_sb = []
        for i in range(8):
            t = sbw.tile([128, 3 * C], BF, tag=f"w{i}", name=f"w{i}")
            nc.sync.dma_start(t[:], wqkv[ts(i, 128), :])
            w_sb.append(t)
        for half in range(2):
            for i in range(8):
                nc.sync.dma_start(xt_sb[i][:, ts(half, 1024)],
                                  xT[ts(i, 128), ts(half, 1024)])
        wo_sb = []
        for i in range(2):
            t = sbw.tile([128, D], BF, tag=f"wo{i}", name=f"wo{i}")
            nc.sync.dma_start(t[:], wo[ts(i, 128), :])
            wo_sb.append(t)
        w_off = {"q": 0, "k": C, "v": 2 * C}

        # ---- persistent activations ----
        qT_sb = [sba.tile([128, seq], BF, tag=f"qT{i}", name=f"qT{i}") for i in range(2)]
        kT_sb = [sba.tile([128, seq], BF, tag=f"kT{i}", name=f"kT{i}") for i in range(2)]
        v_sb = [sba.tile([128, NH * 65], BF, tag=f"v{i}", name=f"v{i}") for i in range(nst)]
        uT_sb = [sba.tile([128, seq], F32, tag=f"uT{i}", name=f"uT{i}") for i in range(2)]
        aT_sb = [sba.tile([128, seq], BF, tag=f"aT{i}", name=f"aT{i}") for i in range(2)]
        rn = [sba.tile([1, seq], F32, tag=f"rn{h}", name=f"rn{h}")
              for h in range(NH)]

        # ---- emission helpers (each emits ~one PSUM tile of PE work) ----
        def emit_qk(name, bias_col, mt, nb):
            dst = qT_sb if name == "q" else kT_sb
            ps = smp.tile([128, 512], F32, tag="sm", name="ps")
            for kt in range(8):
                nc.tensor.matmul(
                    ps[:],
                    lhsT=w_sb[kt][:, ds(w_off[name] + mt * 128, 128)],
                    rhs=xt_sb[kt][:, ts(nb, 512)],
                    start=(kt == 0), stop=(kt == 7))
            nc.vector.tensor_scalar_add(
                dst[mt][:, ts(nb, 512)], ps[:],
                bqk_sb[:, bias_col + mt:bias_col + mt + 1])

        def emit_v(st):
            ps = smp.tile([128, C], F32, tag="sm", name="vps")
            for kt in range(8):
                nc.tensor.matmul(
                    ps[:], lhsT=xt_sb[kt][:, ts(st, 128)],
                    rhs=w_sb[kt][:, ds(2 * C, C)],
                    start=(kt == 0), stop=(kt == 7))
            v3 = v_sb[st][:].rearrange("p (h e) -> p h e", e=65)
            nc.vector.tensor_copy(
                v3[:, :, 64:65],
                ones4[:].rearrange("p (h e) -> p h e", e=1))
            nc.vector.tensor_add(
                v3[:, :, 0:64],
                ps[:].rearrange("p (h e) -> p h e", e=64),
                bvb[:].rearrange("p (h e) -> p h e", e=64))

        def emit_out(qh, i, alt=False):
            mt, nbl = divmod(i, qh_b)
            nb = qh * qh_b + nbl
            yp = smp.tile([128, 512], F32, tag="sm", name="yp")
            for kt2 in range(2):
                nc.tensor.matmul(
                    yp[:], lhsT=wo_sb[kt2][:, ts(mt, 128)],
                    rhs=aT_sb[kt2][:, ts(nb, 512)],
                    start=(kt2 == 0), stop=(kt2 == 1))
            yt = syp.tile([128, 512], F32, tag="yt", name="yt")
            nc.vector.tensor_copy(yt[:], yp[:])
            nc.sync.dma_start(yT[ts(mt, 128), ts(nb, 512)], yt[:])

        def attn_head(qh, h, fillers=()):
            fillers = list(fillers)
            tidx, poff = h // 2, 64 * (h % 2)
            qt, ktt = qT_sb[tidx], kT_sb[tidx]
            pv = pvp.tile([65, qh_w], F32, tag="pv", name="pv")
            pts = {}

            def emit_sc(kt):
                sc = scp.tile([128, qh_w], F32, tag="sc", name="sc")
                for qb in range(qh_b):
                    nc.tensor.matmul(
                        sc[:, ts(qb, 512)],
                        lhsT=ktt[poff:poff + 64, ts(kt, 128)],
                        rhs=qt[poff:poff + 64, ds(qh * qh_w + qb * 512, 512)],
                        start=True, stop=True)
                pt = ptp.tile([128, qh_w], BF, tag="pt", name="pt")
                nc.scalar.activation(pt[:], sc[:], ACT.Exp)
                pts[kt] = pt

            emit_sc(0)
            emit_sc(1)
            fi = 0
            for kt in range(nst):
                # filler first: the PE chews it while waiting on exp(kt)
                n_due = (len(fillers) * (kt + 1) + nst - 1) // nst
                while fi < n_due:
                    fillers[fi]()
                    fi += 1
                if kt + 2 < nst:
                    emit_sc(kt + 2)
                pt = pts.pop(kt)
                for qb in range(qh_b):
                    nc.tensor.matmul(
                        pv[:, ts(qb, 512)],
                        lhsT=v_sb[kt][:, ds(65 * h, 65)],
                        rhs=pt[:, ts(qb, 512)],
                        start=(kt == 0), stop=(kt == nst - 1))
            while fi < len(fillers):
                fillers[fi]()
                fi += 1
            # drain pv: denominator chain first (it gates broadcast->mul->out),
            # then the unnormalized-out copy.  reciprocal_approx_fast misreads
            # PSUM inputs, so stage the denominator row through SBUF.
            dr = sba.tile([1, qh_w], F32, tag=f"dr{h}", name=f"dr{h}")
            nc.vector.tensor_copy(dr[:], pv[64:65, :])
            nc.vector.reciprocal_approx_fast(
                rn[h][:, ds(qh * qh_w, qh_w)], dr[:])
            nc.vector.tensor_copy(
                uT_sb[tidx][poff:poff + 64, ds(qh * qh_w, qh_w)], pv[0:64, :])

        def norm_head(qh, h):
            # broadcast to all 128 partitions (partition_broadcast only
            # writes starting at partition 0); the mul then reads the 64-row
            # half at the same base partition as uT/aT (engine constraint:
            # SBUF-SBUF TensorTensor needs equal base partitions)
            tidx, j = h // 2, h % 2
            sl = ds(qh * qh_w, qh_w)
            rb = rbcp.tile([128, qh_w], F32, tag=f"rb{j}", name=f"rb{j}")
            nc.gpsimd.partition_broadcast(rb[:], rn[h][:, sl])
            half = slice(64 * j, 64 * j + 64)
            nc.vector.tensor_mul(
                aT_sb[tidx][half, sl], uT_sb[tidx][half, sl], rb[half, :])

        # ---------------- schedule ----------------
        for nb in range(nqb):
            emit_qk("q", 0, 0, nb)
        for nb in range(nqb):
            emit_qk("k", 2, 0, nb)
        attn_head(0, 0, [lambda st=st: emit_v(st) for st in range(nst)])
        norm_head(0, 0)
        attn_head(0, 1,
                  [lambda nb=nb: emit_qk("q", 0, 1, nb) for nb in range(nqb)] +
                  [lambda nb=nb: emit_qk("k", 2, 1, nb) for nb in range(nqb)])
        norm_head(0, 1)
        attn_head(0, 2)
        norm_head(0, 2)
        attn_head(0, 3)
        norm_head(0, 3)
        attn_head(1, 0, [lambda i=i: emit_out(0, i) for i in range(0, 4)])
        norm_head(1, 0)
        attn_head(1, 1, [lambda i=i: emit_out(0, i) for i in range(4, 8)])
        norm_head(1, 1)
        attn_head(1, 2, [lambda i=i: emit_out(0, i) for i in range(8, 12)])
        norm_head(1, 2)
        attn_head(1, 3, [lambda i=i: emit_out(0, i) for i in range(12, 16)])
        norm_head(1, 3)
        for i in range(16):
            emit_out(1, i, alt=True)

        if dbg:
            dpool = ctx.enter_context(tc.tile_pool(name="dbg", bufs=2))
            def dump(dram, tiles, width):
                for i, t in enumerate(tiles):
                    for c0 in range(0, width, 1024):
                        w = min(1024, width - c0)
                        dt_ = dpool.tile([128, 1024], F32, tag="d", name="dt")
                        nc.vector.tensor_copy(dt_[:, :w], t[:, c0:c0 + w])
                        nc.sync.dma_start(dram[ts(i, 128), c0:c0 + w],
                                          dt_[:, :w])
            dump(d_qT, qT_sb, seq)
            dump(d_kT, kT_sb, seq)
            dump(d_v, v_sb, NH * 65)
            dump(d_uT, uT_sb, seq)
            dump(d_aT, aT_sb, seq)
            for h in range(NH):
                dt_ = dpool.tile([1, seq], F32, tag="dr", name="dtr")
                nc.vector.tensor_copy(dt_[:], rn[h][:])
                nc.sync.dma_start(d_rn[h:h + 1, :], dt_[:])

    nc.compile()
    return nc


def make_in_maps(x, Wq, bq, Wk, bk, Wv, bv, Wo):
    """Shard full inputs into 8 per-core input maps."""
    import ml_dtypes
    BF = ml_dtypes.bfloat16
    scale = np.float32(1.0 / np.sqrt(DH))
    xT = [np.ascontiguousarray(x[b].T).astype(BF) for b in range(2)]
    in_maps = []
    for c in range(8):
        b, g = c // 4, c % 4
        sl = slice(C * g, C * (g + 1))
        bq_g = (bq[sl] * scale).reshape(2, 128).T
        bk_g = bk[sl].reshape(2, 128).T
        in_maps.append({
            "xT": xT[b],
            "wqkv": np.ascontiguousarray(np.concatenate(
                [Wq[:, sl] * scale, Wk[:, sl], Wv[:, sl]], axis=1)).astype(BF),
            "wo": np.ascontiguousarray(Wo[sl, :]).astype(BF),
            "bqk": np.ascontiguousarray(
                np.concatenate([bq_g, bk_g], axis=1)).astype(np.float32),
            "bv": bv[sl].reshape(1, C).astype(np.float32),
        })
    return in_maps


def kernel(x, Wq, bq, Wk, bk, Wv, bv, Wo, bo):
    if os.environ.get("JAX_PLATFORMS") and \
            "axon" not in os.environ["JAX_PLATFORMS"]:
        os.environ.pop("JAX_PLATFORMS")
    trace = bool(os.environ.get("KERNEL_TRACE"))
    if trace:
        _install_ntff_shim()
    from concourse import bass_utils

    x = np.asarray(x, dtype=np.float32)
    in_maps = make_in_maps(
        x, np.asarray(Wq), np.asarray(bq), np.asarray(Wk), np.asarray(bk),
        np.asarray(Wv), np.asarray(bv), np.asarray(Wo))

    if "nc" not in _CACHE:
        _CACHE["nc"] = build_nc()
    res = bass_utils.run_bass_kernel_spmd(
        _CACHE["nc"], in_maps, core_ids=list(range(8)), trace=trace)
    _CACHE["exec_time_ns"] = res.exec_time_ns

    bo = np.asarray(bo, dtype=np.float32)
    out = np.empty((2, S, D), dtype=np.float32)
    for b in range(2):
        acc = res.results[4 * b]["yT"].copy()
        for g in range(1, 4):
            acc += res.results[4 * b + g]["yT"]
        out[b] = acc.T + bo
    return out
